# revision 13
# baseline (speedup 1.0000x reference)
"""Trainium2 kernel for nn_AttentionModel_PCA (embedding_lookup).

Math: with sf = softmax(Q^T K) per head,
  G[i,m,a] = sum_h sum_j sf[h,i,j] * V[h,a,Z2[j,m]]
           = sum_{(j,c)} T[(j,c),(i,a)] * E[(j,c),m]
where T[(j,c),(i,a)] = sum_h sf[h,i,j] V[h,a,c]  (tiny H=8 contraction)
and E is the one-hot expansion of Z2. The (5376 x 5376) @ (5376 x M)
GEMM producing G is the dominant cost and runs on the 8 NeuronCores
with M sharded (512 samples per core). Host does the cheap prep
(softmax, T, E) and the small tail (take_along_axis, logsumexp,
weighted sums, regularizer).

Device GEMM runs in fp8e4m3 with DoubleRow perf mode (2 fp8 weights
per PE cell -> 256-deep contraction per pass) or bf16. fp32 matmul
streams at 1/4 rate on TRN2, so fp8-DR is ~6x PE throughput vs the
fp32 version; final-scalar error from fp8 T quantization is ~5e-4
(the one-hot E is exact, PSUM accumulates fp32, and the logsumexp
tail averages out per-element noise).
"""

import sys

import numpy as np
import ml_dtypes

for _p in ("/opt/trn_rl_repo",):
    if _p not in sys.path:
        sys.path.append(_p)

H, d, N1, N2, q1, q2, M = 8, 64, 256, 256, 21, 21, 4096
NCORES = 8
MS = M // NCORES          # 512 samples per core
KDIM = N2 * q2            # 5376 contraction (j,c)
PDIM = N1 * q1            # 5376 output rows (i,a)
KT = KDIM // 128          # 42 contraction tiles (bf16 mode)
KT2 = KDIM // 256         # 21 double-row contraction tiles (fp8 mode)
PT = PDIM // 128          # 42 output-row tiles
NS = 6                    # T-slab ring slots
NB = 8                    # PSUM banks ping-ponged
NOT = 6                   # output ring slots
LAMBD = 0.001

MODE = "fp8dr"            # "fp8dr" | "bf16"
FP8_SCALE = 8.0           # |T|max ~16 -> scaled max ~128 < 240 (e4m3 legacy max)

_PROGRAMS = {}


def _build_program(mode):
    """Raw bass pipeline: explicit standalone wait_ge + then_inc.

    sync  : E load, then 42 per-p-tile T slab loads (NS-slot ring)
    tensor: 42 p-tiles x (21 double-row | 42 plain) accumulating matmuls,
            NB PSUM banks round-robin
    vector: PSUM -> SBUF copies (NOT-slot out ring, gated by stores)
    scalar: SBUF -> G stores (HWDGE)
    """
    import concourse.bass as bass
    import concourse.mybir as mybir

    nc = bass.Bass()
    f32 = mybir.dt.float32
    if mode == "fp8dr":
        mdt = mybir.dt.float8e4
        kt, ko = KT2, 2
        perf = mybir.MatmulPerfMode.DoubleRow
    else:
        mdt = mybir.dt.bfloat16
        kt, ko = KT, 1
        perf = None
    kbytes = kt * ko * 128  # free elems per partition of one T slab

    # host-preswizzled layouts (partition dim first, fully contiguous):
    #   Tt[p][ki][t][o][pp] : lhsT tile for (p, t) = [128, (o), 128]
    #   E [ki][t][o][m]     : rhs tile for t = [128, (o), MS]
    Tt = nc.declare_dram_parameter("Tt", [PT, 128, kt * ko * 128], mdt,
                                   isOutput=False)
    E = nc.declare_dram_parameter("E", [128, kt * ko * MS], mdt, isOutput=False)
    G = nc.declare_dram_parameter("G", [PDIM, MS], f32, isOutput=True)

    from contextlib import ExitStack

    ec = 3 if kt % 3 == 0 else 6          # E supertiles per chunk
    nec = kt // ec                        # number of E chunks

    with ExitStack() as stack:
        ent = stack.enter_context
        E_sb = ent(nc.sbuf_tensor([128, kt, ko, MS], mdt))
        slab = ent(nc.sbuf_tensor([128, NS, kt, ko, 128], mdt))
        ot = ent(nc.sbuf_tensor([128, NOT, MS], f32))
        acc = ent(nc.psum_tensor([128, NB * MS], f32))
        # One semaphore per E chunk / ring slot: a DMA's 16 SDMA engines
        # increment its semaphore independently, so with >1 DMA in flight
        # on one shared counting sem, partial sums from two transfers can
        # fake a full count while neither has fully landed. Per-slot sems
        # make every wait exact.
        e_sems = [ent(nc.semaphore(f"e{c}")) for c in range(nec)]
        sl_sems = [ent(nc.semaphore(f"sl{s}")) for s in range(NS)]
        st_sems = [ent(nc.semaphore(f"st{s}")) for s in range(NOT)]
        pe_cnt = ent(nc.semaphore("pe_cnt"))   # p-tiles fully consumed by PE
        cp_sem = ent(nc.semaphore("cp_sem"))   # PSUM->SBUF copies done
        block = ent(nc.Block())

        @block.sync
        def _(sync):
            # slab0 first so the PE can start as soon as E chunk 0 lands;
            # E is chunked so p-tile 0 streams right behind the E DMA, and
            # further slab prefetch is held until E has fully landed so it
            # doesn't steal DMA bandwidth from the critical-path E load.
            # Throttled head: the SDMA engines round-robin across queues at
            # packet granularity, so every in-flight transfer shares the
            # bandwidth fairly. Keep only ~2 in flight, issued in the order
            # the PE needs them, so slab0+E0 land at line rate instead of
            # 1/8 rate.
            def echunk(c):
                return sync.dma_start(
                    E_sb[:, c * ec:(c + 1) * ec],
                    E[:, c * ec * ko * MS:(c + 1) * ec * ko * MS],
                ).then_inc(e_sems[c], 16)

            sync.dma_start(slab[:, 0], Tt[0, :, :]).then_inc(sl_sems[0], 16)
            echunk(0)
            sync.wait_ge(sl_sems[0], 16)
            echunk(1)
            for c in range(2, nec):
                sync.wait_ge(e_sems[c - 2], 16)
                echunk(c)
            sync.wait_ge(e_sems[nec - 2], 16)
            sync.dma_start(slab[:, 1], Tt[1, :, :]).then_inc(sl_sems[1], 16)
            sync.wait_ge(e_sems[nec - 1], 16)
            for p in range(2, PT):
                if p >= NS:
                    sync.wait_ge(pe_cnt, p - NS + 1)
                sync.dma_start(slab[:, p % NS], Tt[p, :, :]
                               ).then_inc(sl_sems[p % NS], 16)

        @block.tensor
        def _(tensor):
            for p in range(PT):
                tensor.wait_ge(sl_sems[p % NS], 16 * (p // NS + 1))
                if p == 1:
                    # belt-and-braces: all E landed before unguarded p-tiles
                    for c in range(nec):
                        tensor.wait_ge(e_sems[c], 16)
                if p >= NB:
                    # bank reused from p-NB: wait for its copy-out
                    tensor.wait_ge(cp_sem, p - NB + 1)
                b = (p % NB) * MS
                for t in range(kt):
                    if p == 0 and t % ec == 0:
                        tensor.wait_ge(e_sems[t // ec], 16)
                    if mode == "fp8dr":
                        lhsT = slab[:, p % NS, t, :, :]
                        rhs = E_sb[:, t, :, :]
                    else:
                        lhsT = slab[:, p % NS, t, 0, :]
                        rhs = E_sb[:, t, 0, :]
                    mm = nc.tensor.matmul(
                        acc[:, b:b + MS], lhsT, rhs,
                        start=(t == 0), stop=(t == kt - 1), perf_mode=perf,
                    )
                    if t == kt - 1:
                        mm.then_inc(pe_cnt, 1)

        @block.vector
        def _(vector):
            for p in range(PT):
                vector.wait_ge(pe_cnt, p + 1)
                if p >= NOT:
                    # ot slot reused from p-NOT: wait for its store
                    vector.wait_ge(st_sems[p % NOT], 16 * (p // NOT))
                nc.vector.tensor_copy(
                    ot[:, p % NOT, :], acc[:, (p % NB) * MS:(p % NB + 1) * MS],
                ).then_inc(cp_sem, 1)

        @block.scalar
        def _(scalar):
            for p in range(PT):
                scalar.wait_ge(cp_sem, p + 1)
                scalar.dma_start(
                    G[p * 128:(p + 1) * 128, :], ot[:, p % NOT, :]
                ).then_inc(st_sems[p % NOT], 16)

    return nc


def host_prep(Q, K, V, Z2):
    """softmax, T (preswizzled + quantized for the PE), one-hot row ids."""
    e = np.einsum("hdi,hdj->hij", Q, K, optimize=True)
    e -= e.max(axis=2, keepdims=True)
    np.exp(e, out=e)
    sf = e / e.sum(axis=2, keepdims=True)
    Tt = np.einsum("hij,hac->jcia", sf, V, optimize=True).reshape(KDIM, PDIM)
    Tt = np.ascontiguousarray(Tt, np.float32)

    if MODE == "fp8dr":
        # Tb[p, ki, t2, o, pp] = s*T[t2*256 + o*128 + ki, p*128 + pp]
        Tq = (Tt * FP8_SCALE).astype(ml_dtypes.float8_e4m3)
        Tb = np.ascontiguousarray(
            Tq.reshape(KT2, 2, 128, PT, 128).transpose(3, 2, 0, 1, 4)
        ).reshape(PT, 128, KT2 * 2 * 128)
        Tdeq = Tq.astype(np.float32) / FP8_SCALE   # for spot checks
    else:
        Tq = Tt.astype(ml_dtypes.bfloat16)
        Tb = np.ascontiguousarray(
            Tq.reshape(KT, 128, PT, 128).transpose(2, 1, 0, 3)
        ).reshape(PT, 128, KT * 128)
        Tdeq = Tq.astype(np.float32)

    # one-hot row index per (j, m): k = j*q2 + Z2[j,m]
    rows = (np.arange(N2, dtype=np.int64)[:, None] * q2 + Z2.astype(np.int64))
    return sf, Tb, Tdeq, rows


def build_E(rows_c):
    """Per-core one-hot E in the device layout [128, kt*ko*MS]."""
    Mloc = rows_c.shape[1]
    dt = ml_dtypes.float8_e4m3 if MODE == "fp8dr" else ml_dtypes.bfloat16
    Eoh = np.zeros((KDIM, Mloc), dt)
    Eoh[rows_c, np.arange(Mloc, dtype=np.int64)[None, :]] = 1.0
    if MODE == "fp8dr":
        Eb = np.ascontiguousarray(
            Eoh.reshape(KT2, 2, 128, Mloc).transpose(2, 0, 1, 3))
    else:
        Eb = np.ascontiguousarray(Eoh.reshape(KT, 128, Mloc).transpose(1, 0, 2))
    return Eb.reshape(128, -1)


def host_tail(G, sf, V, Z1, weights):
    """take_along_axis + logsumexp + loss + regularizer on (N1, M, q1) G."""
    Z1i = Z1.astype(np.int64)
    mat_ene_sum = np.take_along_axis(G, Z1i[:, :, None], axis=2)[..., 0].sum(axis=0)

    Gm = G.max(axis=0)                                   # (M, q1)
    L = np.log(np.exp(G - Gm).sum(axis=0)) + Gm          # (M, q1)
    mx = np.maximum(L.max(axis=1), 0.0)
    logZ = np.log(np.exp(L - mx[:, None]).sum(axis=1)
                  + (N1 - q1) * np.exp(-mx)) + mx

    pl = -(weights.astype(np.float64)
           * (mat_ene_sum.astype(np.float64) - logZ.astype(np.float64))).sum()

    sf2 = sf.reshape(H, -1).astype(np.float64)
    VV = V.reshape(H, -1).astype(np.float64)
    reg = LAMBD * ((sf2 @ sf2.T) * (VV @ VV.T)).sum()
    return np.array(pl + reg, dtype=np.float32)


def run_device(Tb, rows, trace=False, **kw):
    from concourse.bass_utils import run_bass_kernel_spmd

    if MODE not in _PROGRAMS:
        _PROGRAMS[MODE] = _build_program(MODE)
    in_maps = [
        {"Tt": Tb, "E": build_E(rows[:, c * MS:(c + 1) * MS])}
        for c in range(NCORES)
    ]
    out = run_bass_kernel_spmd(_PROGRAMS[MODE], in_maps, list(range(NCORES)),
                               trace=trace, **kw)
    Gf = np.concatenate([np.asarray(out.results[c]["G"]) for c in range(NCORES)],
                        axis=1)                          # (PDIM, M)
    if MODE == "fp8dr":
        Gf = Gf / FP8_SCALE
    return Gf, out


def kernel(**inputs):
    Q = np.asarray(inputs["Q"], np.float32)
    K = np.asarray(inputs["K"], np.float32)
    V = np.asarray(inputs["V"], np.float32)
    Z1 = np.asarray(inputs["Z1"])
    Z2 = np.asarray(inputs["Z2"])
    weights = np.asarray(inputs["weights"], np.float32)

    sf, Tb, _, rows = host_prep(Q, K, V, Z2)
    Gf, _ = run_device(Tb, rows)
    G = Gf.reshape(N1, q1, M).transpose(0, 2, 1)         # (N1, M, q1)
    return host_tail(G, sf, V, Z1, weights)


# revision 15
# speedup vs baseline: 1.0290x; 1.0290x over previous
"""Trainium2 kernel for nn_AttentionModel_PCA (embedding_lookup).

Math: with sf = softmax(Q^T K) per head,
  G[i,m,a] = sum_h sum_j sf[h,i,j] * V[h,a,Z2[j,m]]
           = sum_{(j,c)} T[(j,c),(i,a)] * E[(j,c),m]
where T[(j,c),(i,a)] = sum_h sf[h,i,j] V[h,a,c]  (tiny H=8 contraction)
and E is the one-hot expansion of Z2. The (5376 x 5376) @ (5376 x M)
GEMM producing G is the dominant cost and runs on the 8 NeuronCores
with M sharded (512 samples per core). Host does the cheap prep
(softmax, T, E) and the small tail (take_along_axis, logsumexp,
weighted sums, regularizer).

Device GEMM runs in fp8e4m3 with DoubleRow perf mode (2 fp8 weights
per PE cell -> 256-deep contraction per pass) or bf16. fp32 matmul
streams at 1/4 rate on TRN2, so fp8-DR is ~6x PE throughput vs the
fp32 version; final-scalar error from fp8 T quantization is ~5e-4
(the one-hot E is exact, PSUM accumulates fp32, and the logsumexp
tail averages out per-element noise).
"""

import sys

import numpy as np
import ml_dtypes

for _p in ("/opt/trn_rl_repo",):
    if _p not in sys.path:
        sys.path.append(_p)

H, d, N1, N2, q1, q2, M = 8, 64, 256, 256, 21, 21, 4096
NCORES = 8
MS = M // NCORES          # 512 samples per core
KDIM = N2 * q2            # 5376 contraction (j,c)
PDIM = N1 * q1            # 5376 output rows (i,a)
KT = KDIM // 128          # 42 contraction tiles (bf16 mode)
KT2 = KDIM // 256         # 21 double-row contraction tiles (fp8 mode)
PT = PDIM // 128          # 42 output-row tiles
NS = 6                    # T-slab ring slots
NB = 8                    # PSUM banks ping-ponged
NOT = 6                   # output ring slots
LAMBD = 0.001

MODE = "fp8dr"            # "fp8dr" | "bf16"
FP8_SCALE = 8.0           # |T|max ~16 -> scaled max ~128 < 240 (e4m3 legacy max)

_PROGRAMS = {}


def _build_program(mode):
    """Raw bass pipeline: explicit standalone wait_ge + then_inc.

    sync  : E load, then 42 per-p-tile T slab loads (NS-slot ring)
    tensor: 42 p-tiles x (21 double-row | 42 plain) accumulating matmuls,
            NB PSUM banks round-robin
    vector: PSUM -> SBUF copies (NOT-slot out ring, gated by stores)
    scalar: SBUF -> G stores (HWDGE)
    """
    import concourse.bass as bass
    import concourse.mybir as mybir

    nc = bass.Bass()
    f32 = mybir.dt.float32
    if mode == "fp8dr":
        mdt = mybir.dt.float8e4
        kt, ko = KT2, 2
        perf = mybir.MatmulPerfMode.DoubleRow
    else:
        mdt = mybir.dt.bfloat16
        kt, ko = KT, 1
        perf = None
    kbytes = kt * ko * 128  # free elems per partition of one T slab

    # host-preswizzled layouts (partition dim first, fully contiguous):
    #   Tt[p][ki][t][o][pp] : lhsT tile for (p, t) = [128, (o), 128]
    #   E [ki][t][o][m]     : rhs tile for t = [128, (o), MS]
    Tt = nc.declare_dram_parameter("Tt", [PT, 128, kt * ko * 128], mdt,
                                   isOutput=False)
    E = nc.declare_dram_parameter("E", [128, kt * ko * MS], mdt, isOutput=False)
    G = nc.declare_dram_parameter("G", [PDIM, MS], f32, isOutput=True)

    from contextlib import ExitStack

    ec = 3 if kt % 3 == 0 else 6          # E supertiles per chunk
    nec = kt // ec                        # number of E chunks

    with ExitStack() as stack:
        ent = stack.enter_context
        E_sb = ent(nc.sbuf_tensor([128, kt, ko, MS], mdt))
        slab = ent(nc.sbuf_tensor([128, NS, kt, ko, 128], mdt))
        ot = ent(nc.sbuf_tensor([128, NOT, MS], f32))
        acc = ent(nc.psum_tensor([128, NB * MS], f32))
        # One semaphore per E chunk / ring slot: a DMA's 16 SDMA engines
        # increment its semaphore independently, so with >1 DMA in flight
        # on one shared counting sem, partial sums from two transfers can
        # fake a full count while neither has fully landed. Per-slot sems
        # make every wait exact.
        e_sems = [ent(nc.semaphore(f"e{c}")) for c in range(nec)]
        sl_sems = [ent(nc.semaphore(f"sl{s}")) for s in range(NS)]
        st_sems = [ent(nc.semaphore(f"st{s}")) for s in range(NOT)]
        pe_cnt = ent(nc.semaphore("pe_cnt"))   # p-tiles fully consumed by PE
        cp_sem = ent(nc.semaphore("cp_sem"))   # PSUM->SBUF copies done
        block = ent(nc.Block())

        @block.sync
        def _(sync):
            # slab0 first so the PE can start as soon as E chunk 0 lands;
            # E is chunked so p-tile 0 streams right behind the E DMA, and
            # further slab prefetch is held until E has fully landed so it
            # doesn't steal DMA bandwidth from the critical-path E load.
            # Head: issue slab0/1 + all E chunks concurrently. The SDMA
            # engines round-robin across in-flight transfers, so total
            # completion is bandwidth-optimal (~13us for 4.1 MB); chunked
            # E lets p-tile 0 start as completions trickle in.
            sync.dma_start(slab[:, 0], Tt[0, :, :]).then_inc(sl_sems[0], 16)
            for c in range(nec):
                sync.dma_start(
                    E_sb[:, c * ec:(c + 1) * ec],
                    E[:, c * ec * ko * MS:(c + 1) * ec * ko * MS],
                ).then_inc(e_sems[c], 16)
                if c == 2:
                    sync.dma_start(slab[:, 1], Tt[1, :, :]
                                   ).then_inc(sl_sems[1], 16)
            for c in range(nec):
                sync.wait_ge(e_sems[c], 16)
            for p in range(2, PT):
                if p >= NS:
                    sync.wait_ge(pe_cnt, p - NS + 1)
                sync.dma_start(slab[:, p % NS], Tt[p, :, :]
                               ).then_inc(sl_sems[p % NS], 16)

        def gates(tensor, q):
            """Dependency waits that must hold before p-tile q's matmuls."""
            tensor.wait_ge(sl_sems[q % NS], 16 * (q // NS + 1))
            if q == 1:
                # belt-and-braces: all E landed before unguarded p-tiles
                for c in range(nec):
                    tensor.wait_ge(e_sems[c], 16)
            if q >= NB:
                # bank reused from q-NB: wait for its copy-out
                tensor.wait_ge(cp_sem, q - NB + 1)

        @block.tensor
        def _(tensor):
            gates(tensor, 0)
            for p in range(PT):
                b = (p % NB) * MS
                for t in range(kt):
                    if p == 0 and t % ec == 0:
                        tensor.wait_ge(e_sems[t // ec], 16)
                    if t == kt - 1 and p + 1 < PT:
                        # hoist the next p-tile's gates ahead of our last
                        # matmul: they are satisfied by now in steady state,
                        # and the next tile's LDWEIGHTS can then pull ahead
                        # during this matmul instead of stalling one MM slot
                        gates(tensor, p + 1)
                    if mode == "fp8dr":
                        lhsT = slab[:, p % NS, t, :, :]
                        rhs = E_sb[:, t, :, :]
                    else:
                        lhsT = slab[:, p % NS, t, 0, :]
                        rhs = E_sb[:, t, 0, :]
                    mm = nc.tensor.matmul(
                        acc[:, b:b + MS], lhsT, rhs,
                        start=(t == 0), stop=(t == kt - 1), perf_mode=perf,
                    )
                    if t == kt - 1:
                        mm.then_inc(pe_cnt, 1)

        @block.vector
        def _(vector):
            for p in range(PT):
                vector.wait_ge(pe_cnt, p + 1)
                if p >= NOT:
                    # ot slot reused from p-NOT: wait for its store
                    vector.wait_ge(st_sems[p % NOT], 16 * (p // NOT))
                nc.vector.tensor_copy(
                    ot[:, p % NOT, :], acc[:, (p % NB) * MS:(p % NB + 1) * MS],
                ).then_inc(cp_sem, 1)

        @block.scalar
        def _(scalar):
            for p in range(PT):
                scalar.wait_ge(cp_sem, p + 1)
                scalar.dma_start(
                    G[p * 128:(p + 1) * 128, :], ot[:, p % NOT, :]
                ).then_inc(st_sems[p % NOT], 16)

    return nc


def host_prep(Q, K, V, Z2):
    """softmax, T (preswizzled + quantized for the PE), one-hot row ids."""
    e = np.einsum("hdi,hdj->hij", Q, K, optimize=True)
    e -= e.max(axis=2, keepdims=True)
    np.exp(e, out=e)
    sf = e / e.sum(axis=2, keepdims=True)
    Tt = np.einsum("hij,hac->jcia", sf, V, optimize=True).reshape(KDIM, PDIM)
    Tt = np.ascontiguousarray(Tt, np.float32)

    if MODE == "fp8dr":
        # Tb[p, ki, t2, o, pp] = s*T[t2*256 + o*128 + ki, p*128 + pp]
        Tq = (Tt * FP8_SCALE).astype(ml_dtypes.float8_e4m3)
        Tb = np.ascontiguousarray(
            Tq.reshape(KT2, 2, 128, PT, 128).transpose(3, 2, 0, 1, 4)
        ).reshape(PT, 128, KT2 * 2 * 128)
        Tdeq = Tq.astype(np.float32) / FP8_SCALE   # for spot checks
    else:
        Tq = Tt.astype(ml_dtypes.bfloat16)
        Tb = np.ascontiguousarray(
            Tq.reshape(KT, 128, PT, 128).transpose(2, 1, 0, 3)
        ).reshape(PT, 128, KT * 128)
        Tdeq = Tq.astype(np.float32)

    # one-hot row index per (j, m): k = j*q2 + Z2[j,m]
    rows = (np.arange(N2, dtype=np.int64)[:, None] * q2 + Z2.astype(np.int64))
    return sf, Tb, Tdeq, rows


def build_E(rows_c):
    """Per-core one-hot E in the device layout [128, kt*ko*MS]."""
    Mloc = rows_c.shape[1]
    dt = ml_dtypes.float8_e4m3 if MODE == "fp8dr" else ml_dtypes.bfloat16
    Eoh = np.zeros((KDIM, Mloc), dt)
    Eoh[rows_c, np.arange(Mloc, dtype=np.int64)[None, :]] = 1.0
    if MODE == "fp8dr":
        Eb = np.ascontiguousarray(
            Eoh.reshape(KT2, 2, 128, Mloc).transpose(2, 0, 1, 3))
    else:
        Eb = np.ascontiguousarray(Eoh.reshape(KT, 128, Mloc).transpose(1, 0, 2))
    return Eb.reshape(128, -1)


def host_tail(G, sf, V, Z1, weights):
    """take_along_axis + logsumexp + loss + regularizer on (N1, M, q1) G."""
    Z1i = Z1.astype(np.int64)
    mat_ene_sum = np.take_along_axis(G, Z1i[:, :, None], axis=2)[..., 0].sum(axis=0)

    Gm = G.max(axis=0)                                   # (M, q1)
    L = np.log(np.exp(G - Gm).sum(axis=0)) + Gm          # (M, q1)
    mx = np.maximum(L.max(axis=1), 0.0)
    logZ = np.log(np.exp(L - mx[:, None]).sum(axis=1)
                  + (N1 - q1) * np.exp(-mx)) + mx

    pl = -(weights.astype(np.float64)
           * (mat_ene_sum.astype(np.float64) - logZ.astype(np.float64))).sum()

    sf2 = sf.reshape(H, -1).astype(np.float64)
    VV = V.reshape(H, -1).astype(np.float64)
    reg = LAMBD * ((sf2 @ sf2.T) * (VV @ VV.T)).sum()
    return np.array(pl + reg, dtype=np.float32)


def run_device(Tb, rows, trace=False, **kw):
    from concourse.bass_utils import run_bass_kernel_spmd

    if MODE not in _PROGRAMS:
        _PROGRAMS[MODE] = _build_program(MODE)
    in_maps = [
        {"Tt": Tb, "E": build_E(rows[:, c * MS:(c + 1) * MS])}
        for c in range(NCORES)
    ]
    out = run_bass_kernel_spmd(_PROGRAMS[MODE], in_maps, list(range(NCORES)),
                               trace=trace, **kw)
    Gf = np.concatenate([np.asarray(out.results[c]["G"]) for c in range(NCORES)],
                        axis=1)                          # (PDIM, M)
    if MODE == "fp8dr":
        Gf = Gf / FP8_SCALE
    return Gf, out


def kernel(**inputs):
    Q = np.asarray(inputs["Q"], np.float32)
    K = np.asarray(inputs["K"], np.float32)
    V = np.asarray(inputs["V"], np.float32)
    Z1 = np.asarray(inputs["Z1"])
    Z2 = np.asarray(inputs["Z2"])
    weights = np.asarray(inputs["weights"], np.float32)

    sf, Tb, _, rows = host_prep(Q, K, V, Z2)
    Gf, _ = run_device(Tb, rows)
    G = Gf.reshape(N1, q1, M).transpose(0, 2, 1)         # (N1, M, q1)
    return host_tail(G, sf, V, Z1, weights)


# revision 16
# speedup vs baseline: 1.0394x; 1.0101x over previous
"""Trainium2 kernel for nn_AttentionModel_PCA (embedding_lookup).

Math: with sf = softmax(Q^T K) per head,
  G[i,m,a] = sum_h sum_j sf[h,i,j] * V[h,a,Z2[j,m]]
           = sum_{(j,c)} T[(j,c),(i,a)] * E[(j,c),m]
where T[(j,c),(i,a)] = sum_h sf[h,i,j] V[h,a,c]  (tiny H=8 contraction)
and E is the one-hot expansion of Z2. The (5376 x 5376) @ (5376 x M)
GEMM producing G is the dominant cost and runs on the 8 NeuronCores
with M sharded (512 samples per core). Host does the cheap prep
(softmax, T, E) and the small tail (take_along_axis, logsumexp,
weighted sums, regularizer).

Device GEMM runs in fp8e4m3 with DoubleRow perf mode (2 fp8 weights
per PE cell -> 256-deep contraction per pass) or bf16. fp32 matmul
streams at 1/4 rate on TRN2, so fp8-DR is ~6x PE throughput vs the
fp32 version; final-scalar error from fp8 T quantization is ~5e-4
(the one-hot E is exact, PSUM accumulates fp32, and the logsumexp
tail averages out per-element noise).
"""

import sys

import numpy as np
import ml_dtypes

for _p in ("/opt/trn_rl_repo",):
    if _p not in sys.path:
        sys.path.append(_p)

H, d, N1, N2, q1, q2, M = 8, 64, 256, 256, 21, 21, 4096
NCORES = 8
MS = M // NCORES          # 512 samples per core
KDIM = N2 * q2            # 5376 contraction (j,c)
PDIM = N1 * q1            # 5376 output rows (i,a)
KT = KDIM // 128          # 42 contraction tiles (bf16 mode)
KT2 = KDIM // 256         # 21 double-row contraction tiles (fp8 mode)
PT = PDIM // 128          # 42 output-row tiles
NS = 6                    # T-slab ring slots
NB = 8                    # PSUM banks ping-ponged
NOT = 6                   # output ring slots
LAMBD = 0.001

MODE = "fp8dr"            # "fp8dr" | "bf16"
FP8_SCALE = 8.0           # |T|max ~16 -> scaled max ~128 < 240 (e4m3 legacy max)

_PROGRAMS = {}


def _build_program(mode):
    """Raw bass pipeline: explicit standalone wait_ge + then_inc.

    sync  : E load, then 42 per-p-tile T slab loads (NS-slot ring)
    tensor: 42 p-tiles x (21 double-row | 42 plain) accumulating matmuls,
            NB PSUM banks round-robin
    vector: PSUM -> SBUF copies (NOT-slot out ring, gated by stores)
    scalar: SBUF -> G stores (HWDGE)
    """
    import concourse.bass as bass
    import concourse.mybir as mybir

    nc = bass.Bass()
    f32 = mybir.dt.float32
    if mode == "fp8dr":
        mdt = mybir.dt.float8e4
        kt, ko = KT2, 2
        perf = mybir.MatmulPerfMode.DoubleRow
    else:
        mdt = mybir.dt.bfloat16
        kt, ko = KT, 1
        perf = None
    kbytes = kt * ko * 128  # free elems per partition of one T slab

    # host-preswizzled layouts (partition dim first, fully contiguous):
    #   Tt[p][ki][t][o][pp] : lhsT tile for (p, t) = [128, (o), 128]
    #   E [ki][t][o][m]     : rhs tile for t = [128, (o), MS]
    Tt = nc.declare_dram_parameter("Tt", [PT, 128, kt * ko * 128], mdt,
                                   isOutput=False)
    E = nc.declare_dram_parameter("E", [128, kt * ko * MS], mdt, isOutput=False)
    G = nc.declare_dram_parameter("G", [PDIM, MS], f32, isOutput=True)

    from contextlib import ExitStack

    ec = 3 if kt % 3 == 0 else 6          # E supertiles per chunk
    nec = kt // ec                        # number of E chunks

    with ExitStack() as stack:
        ent = stack.enter_context
        E_sb = ent(nc.sbuf_tensor([128, kt, ko, MS], mdt))
        slab = ent(nc.sbuf_tensor([128, NS, kt, ko, 128], mdt))
        ot = ent(nc.sbuf_tensor([128, NOT, MS], f32))
        acc = ent(nc.psum_tensor([128, NB * MS], f32))
        # One semaphore per E chunk / ring slot: a DMA's 16 SDMA engines
        # increment its semaphore independently, so with >1 DMA in flight
        # on one shared counting sem, partial sums from two transfers can
        # fake a full count while neither has fully landed. Per-slot sems
        # make every wait exact.
        e_sems = [ent(nc.semaphore(f"e{c}")) for c in range(nec)]
        sl_sems = [ent(nc.semaphore(f"sl{s}")) for s in range(NS)]
        st_sems = [ent(nc.semaphore(f"st{s}")) for s in range(NOT)]
        pe_cnt = ent(nc.semaphore("pe_cnt"))   # p-tiles fully consumed by PE
        cp_sem = ent(nc.semaphore("cp_sem"))   # PSUM->SBUF copies done
        block = ent(nc.Block())

        @block.sync
        def _(sync):
            # slab0 first so the PE can start as soon as E chunk 0 lands;
            # E is chunked so p-tile 0 streams right behind the E DMA, and
            # further slab prefetch is held until E has fully landed so it
            # doesn't steal DMA bandwidth from the critical-path E load.
            # Head: issue slab0/1 + all E chunks concurrently. The SDMA
            # engines round-robin across in-flight transfers, so total
            # completion is bandwidth-optimal (~13us for 4.1 MB); chunked
            # E lets p-tile 0 start as completions trickle in.
            sync.dma_start(slab[:, 0], Tt[0, :, :]).then_inc(sl_sems[0], 16)
            for c in range(nec):
                sync.dma_start(
                    E_sb[:, c * ec:(c + 1) * ec],
                    E[:, c * ec * ko * MS:(c + 1) * ec * ko * MS],
                ).then_inc(e_sems[c], 16)
                if c == 2:
                    sync.dma_start(slab[:, 1], Tt[1, :, :]
                                   ).then_inc(sl_sems[1], 16)
            for c in range(nec):
                sync.wait_ge(e_sems[c], 16)
            for p in range(2, PT):
                if p >= NS:
                    sync.wait_ge(pe_cnt, p - NS + 1)
                sync.dma_start(slab[:, p % NS], Tt[p, :, :]
                               ).then_inc(sl_sems[p % NS], 16)

        def gates(tensor, q):
            """Dependency waits that must hold before p-tile q's matmuls."""
            tensor.wait_ge(sl_sems[q % NS], 16 * (q // NS + 1))
            if q == 1:
                # belt-and-braces: all E landed before unguarded p-tiles
                for c in range(nec):
                    tensor.wait_ge(e_sems[c], 16)
            if q >= NB:
                # bank reused from q-NB: wait for its copy-out
                tensor.wait_ge(cp_sem, q - NB + 1)

        @block.tensor
        def _(tensor):
            for p in range(PT):
                gates(tensor, p)
                b = (p % NB) * MS
                for t in range(kt):
                    if p == 0 and t % ec == 0:
                        tensor.wait_ge(e_sems[t // ec], 16)
                    if mode == "fp8dr":
                        lhsT = slab[:, p % NS, t, :, :]
                        rhs = E_sb[:, t, :, :]
                    else:
                        lhsT = slab[:, p % NS, t, 0, :]
                        rhs = E_sb[:, t, 0, :]
                    mm = nc.tensor.matmul(
                        acc[:, b:b + MS], lhsT, rhs,
                        start=(t == 0), stop=(t == kt - 1), perf_mode=perf,
                    )
                    if t == kt - 1:
                        mm.then_inc(pe_cnt, 1)

        @block.vector
        def _(vector):
            for p in range(PT):
                vector.wait_ge(pe_cnt, p + 1)
                if p >= NOT:
                    # ot slot reused from p-NOT: wait for its store
                    vector.wait_ge(st_sems[p % NOT], 16 * (p // NOT))
                nc.vector.tensor_copy(
                    ot[:, p % NOT, :], acc[:, (p % NB) * MS:(p % NB + 1) * MS],
                ).then_inc(cp_sem, 1)

        @block.scalar
        def _(scalar):
            for p in range(PT):
                scalar.wait_ge(cp_sem, p + 1)
                scalar.dma_start(
                    G[p * 128:(p + 1) * 128, :], ot[:, p % NOT, :]
                ).then_inc(st_sems[p % NOT], 16)

    return nc


def host_prep(Q, K, V, Z2):
    """softmax, T (preswizzled + quantized for the PE), one-hot row ids."""
    e = np.einsum("hdi,hdj->hij", Q, K, optimize=True)
    e -= e.max(axis=2, keepdims=True)
    np.exp(e, out=e)
    sf = e / e.sum(axis=2, keepdims=True)
    Tt = np.einsum("hij,hac->jcia", sf, V, optimize=True).reshape(KDIM, PDIM)
    Tt = np.ascontiguousarray(Tt, np.float32)

    if MODE == "fp8dr":
        # Tb[p, ki, t2, o, pp] = s*T[t2*256 + o*128 + ki, p*128 + pp]
        Tq = (Tt * FP8_SCALE).astype(ml_dtypes.float8_e4m3)
        Tb = np.ascontiguousarray(
            Tq.reshape(KT2, 2, 128, PT, 128).transpose(3, 2, 0, 1, 4)
        ).reshape(PT, 128, KT2 * 2 * 128)
        Tdeq = Tq.astype(np.float32) / FP8_SCALE   # for spot checks
    else:
        Tq = Tt.astype(ml_dtypes.bfloat16)
        Tb = np.ascontiguousarray(
            Tq.reshape(KT, 128, PT, 128).transpose(2, 1, 0, 3)
        ).reshape(PT, 128, KT * 128)
        Tdeq = Tq.astype(np.float32)

    # one-hot row index per (j, m): k = j*q2 + Z2[j,m]
    rows = (np.arange(N2, dtype=np.int64)[:, None] * q2 + Z2.astype(np.int64))
    return sf, Tb, Tdeq, rows


def build_E(rows_c):
    """Per-core one-hot E in the device layout [128, kt*ko*MS]."""
    Mloc = rows_c.shape[1]
    dt = ml_dtypes.float8_e4m3 if MODE == "fp8dr" else ml_dtypes.bfloat16
    Eoh = np.zeros((KDIM, Mloc), dt)
    Eoh[rows_c, np.arange(Mloc, dtype=np.int64)[None, :]] = 1.0
    if MODE == "fp8dr":
        Eb = np.ascontiguousarray(
            Eoh.reshape(KT2, 2, 128, Mloc).transpose(2, 0, 1, 3))
    else:
        Eb = np.ascontiguousarray(Eoh.reshape(KT, 128, Mloc).transpose(1, 0, 2))
    return Eb.reshape(128, -1)


def host_tail(G, sf, V, Z1, weights):
    """take_along_axis + logsumexp + loss + regularizer on (N1, M, q1) G."""
    Z1i = Z1.astype(np.int64)
    mat_ene_sum = np.take_along_axis(G, Z1i[:, :, None], axis=2)[..., 0].sum(axis=0)

    Gm = G.max(axis=0)                                   # (M, q1)
    L = np.log(np.exp(G - Gm).sum(axis=0)) + Gm          # (M, q1)
    mx = np.maximum(L.max(axis=1), 0.0)
    logZ = np.log(np.exp(L - mx[:, None]).sum(axis=1)
                  + (N1 - q1) * np.exp(-mx)) + mx

    pl = -(weights.astype(np.float64)
           * (mat_ene_sum.astype(np.float64) - logZ.astype(np.float64))).sum()

    sf2 = sf.reshape(H, -1).astype(np.float64)
    VV = V.reshape(H, -1).astype(np.float64)
    reg = LAMBD * ((sf2 @ sf2.T) * (VV @ VV.T)).sum()
    return np.array(pl + reg, dtype=np.float32)


def run_device(Tb, rows, trace=False, **kw):
    from concourse.bass_utils import run_bass_kernel_spmd

    if MODE not in _PROGRAMS:
        _PROGRAMS[MODE] = _build_program(MODE)
    in_maps = [
        {"Tt": Tb, "E": build_E(rows[:, c * MS:(c + 1) * MS])}
        for c in range(NCORES)
    ]
    out = run_bass_kernel_spmd(_PROGRAMS[MODE], in_maps, list(range(NCORES)),
                               trace=trace, **kw)
    Gf = np.concatenate([np.asarray(out.results[c]["G"]) for c in range(NCORES)],
                        axis=1)                          # (PDIM, M)
    if MODE == "fp8dr":
        Gf = Gf / FP8_SCALE
    return Gf, out


def kernel(**inputs):
    Q = np.asarray(inputs["Q"], np.float32)
    K = np.asarray(inputs["K"], np.float32)
    V = np.asarray(inputs["V"], np.float32)
    Z1 = np.asarray(inputs["Z1"])
    Z2 = np.asarray(inputs["Z2"])
    weights = np.asarray(inputs["weights"], np.float32)

    sf, Tb, _, rows = host_prep(Q, K, V, Z2)
    Gf, _ = run_device(Tb, rows)
    G = Gf.reshape(N1, q1, M).transpose(0, 2, 1)         # (N1, M, q1)
    return host_tail(G, sf, V, Z1, weights)
